# revision 5
# baseline (speedup 1.0000x reference)
"""Trainium2 Bass kernel for segment-reduced pairwise L2 distance.

Math: reference computes
    dist[p, n] = sqrt(max(||t_p||^2 - 2 t_p.x_n + ||x_n||^2, 0) + 1e-8)
    out[n]     = mean_s( mean_{p in seg s}( dist[p, n] ) )
which collapses exactly to a weighted sum over positions:
    out[n] = sum_p w_p * dist[p, n],   w_p = 1 / (n_seg * max(count[seg_p], 1))

Device kernel (per core, nodes sharded 8 ways, 6272 padded nodes each):
  psum[n128, p2048] = w2_p*(p2_n + t2_p + eps) - 2*w2_p*cross   via
     (a) K=128 bf16 matmul: predT_tile.T @ (-2*w2*target^T)
     (b) K=4  bf16 matmul adding outer-product bias rows
         lhsT=[p2_hi; p2_lo; 1; 1], rhs=[w2; w2; (w2*(t2+eps))_hi; _lo]
  then one ScalarE Sqrt over [128, 2048] with accum_out giving
  acc[n, tile] = sum_p w_p * dist[p, n] directly. No DVE work at all.
"""

import numpy as np
import ml_dtypes

import concourse.bass as bass
import concourse.tile as tile
from concourse import bacc, mybir
from concourse.bass_utils import run_bass_kernel_spmd

BF16 = ml_dtypes.bfloat16

N_CORES = 8
D = 128
N_POS = 2048
N_NODES = 50000
NODES_PER_CORE = N_NODES // N_CORES       # 6250
N_TILES = 49                              # ceil(6250/128)
NODES_PAD = N_TILES * 128                 # 6272
CHUNK = 512
N_CHUNKS = N_POS // CHUNK                 # 4
PRED_DMA_SPLIT = 7                        # 7 DMA slabs of 896 cols each
SUBW = NODES_PAD // PRED_DMA_SPLIT        # 896 = 7 n-tiles
ACC_COLS = 64                             # acc tile free dim (49 used)
EPS = 1e-8


def build_bass():
    # Bacc (not plain Bass): its compile() runs move_matmul_waits_to_ldweights
    # + generate_event_semaphores, which split multi-wait Matmults that
    # otherwise fail walrus codegen ("Too many sync wait commands").
    nc = bacc.Bacc()
    predT = nc.declare_dram_parameter(
        "predT", [D, NODES_PAD], mybir.dt.bfloat16, isOutput=False)
    augL = nc.declare_dram_parameter(
        "augL", [4, NODES_PAD], mybir.dt.bfloat16, isOutput=False)
    trg = nc.declare_dram_parameter(
        "trg", [D, N_POS], mybir.dt.bfloat16, isOutput=False)
    augR = nc.declare_dram_parameter(
        "augR", [4, N_POS], mybir.dt.bfloat16, isOutput=False)
    outp = nc.declare_dram_parameter(
        "out", [128, ACC_COLS], mybir.dt.float32, isOutput=True)

    with tile.TileContext(nc) as tc:
        with (
            tc.tile_pool(name="consts", bufs=1) as consts,
            tc.tile_pool(name="junk", bufs=2) as junkp,
            tc.tile_pool(name="psum", bufs=2, space="PSUM") as psump,
        ):
            trg_sb = consts.tile([D, N_POS], mybir.dt.bfloat16)
            nc.sync.dma_start(trg_sb[:], trg[:])
            augR_sb = consts.tile([4, N_POS], mybir.dt.bfloat16)
            nc.sync.dma_start(augR_sb[:], augR[:])
            augL_sb = consts.tile([4, NODES_PAD], mybir.dt.bfloat16)
            nc.sync.dma_start(augL_sb[:], augL[:])

            pred_tiles = []
            for s in range(PRED_DMA_SPLIT):
                t = consts.tile([D, SUBW], mybir.dt.bfloat16, tag=f"pred{s}")
                nc.sync.dma_start(t[:], predT[:, s * SUBW:(s + 1) * SUBW])
                pred_tiles.append(t)

            acc = consts.tile([128, ACC_COLS], mybir.dt.float32)

            # Warmup ACT op at kernel start: triggers the ~2.7us sqrt
            # table-set load while the input DMAs stream, instead of on the
            # first real tile's critical path. Result lands in an unused
            # acc column (host reads only the first N_TILES columns).
            warm = consts.tile([128, 1], mybir.dt.float32)
            nc.gpsimd.memset(warm[:], 1.0)
            warm_out = consts.tile([128, 1], mybir.dt.bfloat16)
            nc.scalar.activation(
                warm_out[:], warm[:], mybir.ActivationFunctionType.Sqrt,
                accum_out=acc[:, ACC_COLS - 1:ACC_COLS])

            for ti in range(N_TILES):
                lhs = pred_tiles[ti // 7][:, (ti % 7) * 128:(ti % 7 + 1) * 128]
                ps = psump.tile([128, N_POS], mybir.dt.float32)
                for j in range(N_CHUNKS):
                    nc.tensor.matmul(
                        ps[:, j * CHUNK:(j + 1) * CHUNK],
                        lhsT=lhs,
                        rhs=trg_sb[:, j * CHUNK:(j + 1) * CHUNK],
                        start=True, stop=False)
                for j in range(N_CHUNKS):
                    nc.tensor.matmul(
                        ps[:, j * CHUNK:(j + 1) * CHUNK],
                        lhsT=augL_sb[:, ti * 128:(ti + 1) * 128],
                        rhs=augR_sb[:, j * CHUNK:(j + 1) * CHUNK],
                        start=False, stop=True)
                junk = junkp.tile([128, N_POS], mybir.dt.bfloat16)
                nc.scalar.activation(
                    junk[:], ps[:], mybir.ActivationFunctionType.Sqrt,
                    accum_out=acc[:, ti:ti + 1])

            nc.sync.dma_start(outp[:], acc[:])
    nc.compile()
    return nc


def _bf16_split(a):
    """Return (hi, lo) bf16 arrays with hi+lo ~= a to ~1e-5 rel."""
    a = np.asarray(a, np.float64)
    hi = a.astype(BF16)
    lo = (a - hi.astype(np.float64)).astype(BF16)
    return hi, lo


def prepare_inputs(pred, target, target_identifiers, num_segments):
    """Host-side prep: weights, padding, transposes, bf16 conversion."""
    nseg = int(num_segments)
    tid = np.asarray(target_identifiers).astype(np.int64)
    pred = np.asarray(pred, np.float32)
    target = np.asarray(target, np.float32)

    counts = np.bincount(tid, minlength=nseg).astype(np.float64)
    w = 1.0 / (nseg * np.maximum(counts, 1.0))
    wp = w[tid]                                   # [n_pos]
    w2 = wp * wp

    t2 = (target.astype(np.float64) ** 2).sum(-1)          # [n_pos]
    p2 = (pred.astype(np.float64) ** 2).sum(-1)            # [n_nodes]

    # replicated operands
    trg_np = np.ascontiguousarray(
        (-2.0 * w2[:, None] * target).T).astype(BF16)      # [128, 2048]
    a_hi, a_lo = _bf16_split(w2 * (t2 + EPS))
    augR_np = np.stack([
        w2.astype(BF16), w2.astype(BF16), a_hi, a_lo])     # [4, 2048]

    # per-core operands
    predT_full = pred.T                                    # [128, 50000]
    p2_hi_f, p2_lo_f = _bf16_split(p2)
    in_maps = []
    for c in range(N_CORES):
        sl = slice(c * NODES_PER_CORE, (c + 1) * NODES_PER_CORE)
        pt = np.zeros((D, NODES_PAD), BF16)
        pt[:, :NODES_PER_CORE] = predT_full[:, sl].astype(BF16)
        augL_np = np.zeros((4, NODES_PAD), BF16)
        augL_np[0, :NODES_PER_CORE] = p2_hi_f[sl]
        augL_np[1, :NODES_PER_CORE] = p2_lo_f[sl]
        augL_np[2] = 1.0
        augL_np[3] = 1.0
        in_maps.append({
            "predT": np.ascontiguousarray(pt),
            "augL": np.ascontiguousarray(augL_np),
            "trg": trg_np,
            "augR": augR_np,
        })
    return in_maps


def gather_output(results):
    outs = []
    for c in range(N_CORES):
        blk = np.asarray(results[c]["out"])       # [128, ACC_COLS] f32
        outs.append(blk[:, :N_TILES].T.reshape(-1)[:NODES_PER_CORE])
    return np.concatenate(outs).astype(np.float32)


_CACHE = {}


def kernel(pred, target, target_identifiers, num_segments):
    in_maps = prepare_inputs(pred, target, target_identifiers, num_segments)
    if "nc" not in _CACHE:
        _CACHE["nc"] = build_bass()
    res = run_bass_kernel_spmd(_CACHE["nc"], in_maps, list(range(N_CORES)))
    return gather_output(res.results)
